# revision 9
# baseline (speedup 1.0000x reference)
"""Trainium2 Bass kernel for nn_ClassificationLayer (Gaussian pdf-sum classifier).

Math:
  mu/sd per dim from tiny [128,10] reference sets (host, exact).
  Per row i: s_n[i] = sum_d INV_SQRT_2PI/sd_d * exp(-0.5*((x[i,d]-mu_d)/sd_d)^2)
  (same for anomaly), then the batch recurrence p_k = (p_{k-1} + s_k)/128,
  output = [pn/(pn+pa), pa/(pn+pa)].

Device strategy (8 cores, data-parallel over N):
  - Host transposes each core's row-shard to [128 dims, R rows] fp16 so
    per-dim constants become per-partition scale/bias vectors.
  - The 2*R Gaussian evaluations per core are split between TWO engines:
    * ScalarEngine (~2/3 of row-chunks): one ACTIVATE per distribution,
      Derivative_Erf(scale*x + bias) = (2/sqrt(pi)) * exp(-((x-mu)/sd)^2/2),
      fp16 out. ACTIVATE is 1 col/cycle regardless of dtype, so this engine
      caps at ~105us/core alone -- hence the second path.
    * VectorEngine (~1/3 of row-chunks): Schraudolph fast-exp in 3 DVE ops,
      all in 2x/4x perf modes:
        z' = (A2/sd)*x - A2*mu/sd          (tensor_scalar, fp16, 4x)
        t  = z'*z'                          (tensor_tensor, fp16, 2x)
        u  = round(B - t) -> uint16         (tensor_scalar, 4x)
      with A2 = sqrt((128/ln2)/2) so t = (128/ln2)*(z^2/2). The uint16 is
      the bf16 bit pattern of (2/sqrt(pi))*exp(-z^2/2) (B encodes the bf16
      exponent bias, the 2/sqrt(pi) factor, and a sawtooth-centering shift).
      The fp32->uint16 write converter rounds-to-nearest and SATURATES at
      [0, 65535] (probed on HW), so t > B (deep tail) cleanly becomes +0.0.
      Per-element error is a +-4% sawtooth that cancels in the final ratio.
  - Reduction over dims (partitions) via TensorEngine matvec: the stationary
    operand is a 64-wide shifted window over a zero-padded weight buffer so
    chunk g's sums land in PSUM partition g%64 of bank g//64. ACT chunks use
    an fp16 weight copy, DVE chunks the bf16 copy (moving dtype must match).
  - The scalar recurrence decays by 1/128 per step, so it is re-run exactly
    on the gathered per-row sums on host as a short causal convolution.
"""

import numpy as np

N, DIM, S = 500000, 128, 10
INV_SQRT_2PI = 0.3989422804014327
NCORES = 8
CHUNK = 512                      # rows per matvec (PSUM bank free-dim)
NCHUNK = 123                     # chunks per core  (123*512 = 62976 rows)
R = NCHUNK * CHUNK               # rows per core, 8*R = 503808 >= N

# Schraudolph constants (bf16 bit trick)
A_EXP = 128.0 / np.log(2.0)           # 184.6627
A2 = float(np.sqrt(A_EXP / 2.0))      # 9.60891
B_EXP = float(16256.0 + 128.0 * np.log2(2.0 / np.sqrt(np.pi)) - 5.0)

# Tiles: (width, act_chunks, dve_chunks); act+dve chunks = width/512.
# Small head tiles so ScalarE starts before the first big DMA lands; big
# middle tiles amortize per-instruction + semaphore overhead; small tail
# tiles so the final PSUM drain isn't gated by a huge last tile.
# Split tuned so ScalarE busy (~854ns/chunk) ~= DVE busy (~1100ns/chunk).
TILE_SPEC = [(1024, 1, 1), (3072, 3, 3), (6144, 6, 6),
             (9216, 11, 7), (9216, 10, 8), (9216, 10, 8), (9216, 10, 8),
             (9216, 10, 8), (4096, 4, 4), (2048, 2, 2), (512, 1, 0)]
assert sum(w for w, _, _ in TILE_SPEC) == R
assert all(a + d == w // CHUNK for w, a, d in TILE_SPEC)
MAX_W = max(w for w, _, _ in TILE_SPEC)
MAX_WA = max(a * CHUNK for _, a, _ in TILE_SPEC)
MAX_WD = max(d * CHUNK for _, _, d in TILE_SPEC)

_COMPILED = None
LAST_RESULTS = None  # BassKernelResults of the most recent device run


def _build():
    import concourse.tile as tile
    from concourse import bacc, mybir

    nc = bacc.Bacc("TRN2", target_bir_lowering=False, debug=False,
                   num_devices=NCORES)

    xT = nc.dram_tensor("xT", [DIM, R], mybir.dt.float16,
                        kind="ExternalInput").ap()
    # consts cols: 0 scale_n, 1 bias_n, 2 scale_a, 3 bias_a (ACT path),
    #              4 s'_n, 5 b'_n, 6 s'_a, 7 b'_a (DVE path)
    consts = nc.dram_tensor("consts", [DIM, 8], mybir.dt.float32,
                            kind="ExternalInput").ap()
    # weight windows: col 128-r of window [128-r, 192-r) is the weight
    # vector; c_n at col 128, c_a at col 384, zeros elsewhere.
    wf16 = nc.dram_tensor("wf16", [DIM, 512], mybir.dt.float16,
                          kind="ExternalInput").ap()
    wbf16 = nc.dram_tensor("wbf16", [DIM, 512], mybir.dt.bfloat16,
                           kind="ExternalInput").ap()
    sn_out = nc.dram_tensor("sn_out", [128, CHUNK], mybir.dt.float32,
                            kind="ExternalOutput").ap()
    sa_out = nc.dram_tensor("sa_out", [128, CHUNK], mybir.dt.float32,
                            kind="ExternalOutput").ap()

    DErf = mybir.ActivationFunctionType.Derivative_Erf
    Alu = mybir.AluOpType

    # tile index after which PSUM bank A (chunks 0-63) is complete
    FLUSH_AFTER_TILE = None
    cum = 0
    for ti, (w, _, _) in enumerate(TILE_SPEC):
        cum += w // CHUNK
        if cum >= 64 and FLUSH_AFTER_TILE is None:
            FLUSH_AFTER_TILE = ti

    with tile.TileContext(nc) as tc:
        with tc.tile_pool(name="cpool", bufs=1) as cpool, \
             tc.tile_pool(name="xpool", bufs=3) as xpool, \
             tc.tile_pool(name="epool", bufs=3) as epool, \
             tc.tile_pool(name="dpool", bufs=2) as dpool, \
             tc.tile_pool(name="upool", bufs=3) as upool, \
             tc.tile_pool(name="pspool", bufs=1, space="PSUM") as pspool:

            # consts first: tiny transfer that gates the table-load dummy
            consts_t = cpool.tile([DIM, 8], mybir.dt.float32)
            nc.sync.dma_start(consts_t[:], consts[:, :])
            # weights via SWDGE early so the PE can start as soon as the
            # first activations land; Sync HWDGE queue stays x-tiles only
            wf_t = cpool.tile([DIM, 512], mybir.dt.float16)
            nc.gpsimd.dma_start(wf_t[:], wf16[:, :])
            wb_t = cpool.tile([DIM, 512], mybir.dt.bfloat16)
            nc.gpsimd.dma_start(wb_t[:], wbf16[:, :])
            # prefetch the first x tiles so the first ACTIVATE's data is in
            # flight while the activation table loads
            x_pre = {}
            for ti in (0, 1):
                off = sum(TILE_SPEC[t][0] for t in range(ti))
                w = TILE_SPEC[ti][0]
                x_t = xpool.tile([DIM, w], mybir.dt.float16, tag="x",
                                 padded_shape=[DIM, MAX_W],
                                 name=f"x_pre{ti}")
                nc.sync.dma_start(x_t[:], xT[:, off:off + w])
                x_pre[ti] = x_t
            # Dummy activation: triggers the erf_derivative table load while
            # the first x tile is still in flight.
            warm_t = cpool.tile([DIM, 1], mybir.dt.float32)
            nc.scalar.activation(warm_t[:], consts_t[:, 0:1], DErf,
                                 bias=0.0, scale=1.0)

            # per dist: bank A = chunks 0-63, bank B = chunks 64-122
            sn_psA = pspool.tile([64, CHUNK], mybir.dt.float32)
            sn_psB = pspool.tile([64, CHUNK], mybir.dt.float32)
            sa_psA = pspool.tile([64, CHUNK], mybir.dt.float32)
            sa_psB = pspool.tile([64, CHUNK], mybir.dt.float32)

            def mm(dist_ps, w_t, base, mov, g):
                r = g % 64
                ps = dist_ps[0] if g < 64 else dist_ps[1]
                first = r == 0
                last = g == 63 or g == NCHUNK - 1
                nc.tensor.matmul(ps[:], w_t[:, base - r:base + 64 - r], mov,
                                 start=first, stop=last,
                                 skip_group_check=True)

            g = 0
            off = 0
            for ti, (w, ca, cd) in enumerate(TILE_SPEC):
                if ti in x_pre:
                    x_t = x_pre[ti]
                else:
                    x_t = xpool.tile([DIM, w], mybir.dt.float16, tag="x",
                                     padded_shape=[DIM, MAX_W])
                    nc.sync.dma_start(x_t[:], xT[:, off:off + w])
                wa = ca * CHUNK
                wd = cd * CHUNK

                # --- ScalarE path: cols [0, wa) ---
                en_t = epool.tile([DIM, wa], mybir.dt.float16, tag="en",
                                  padded_shape=[DIM, MAX_WA])
                nc.scalar.activation(en_t[:], x_t[:, 0:wa], DErf,
                                     bias=consts_t[:, 1:2],
                                     scale=consts_t[:, 0:1])
                ea_t = epool.tile([DIM, wa], mybir.dt.float16, tag="ea",
                                  padded_shape=[DIM, MAX_WA])
                nc.scalar.activation(ea_t[:], x_t[:, 0:wa], DErf,
                                     bias=consts_t[:, 3:4],
                                     scale=consts_t[:, 2:3])

                # --- DVE path: cols [wa, w) ---
                if wd:
                    xd = x_t[:, wa:w]
                    un_t = None
                    ua_t = None
                    for dist in ("n", "a"):
                        sc = (4, 5) if dist == "n" else (6, 7)
                        z_t = dpool.tile([DIM, wd], mybir.dt.float16,
                                         tag="z", padded_shape=[DIM, MAX_WD])
                        nc.vector.tensor_scalar(z_t[:], xd,
                                                consts_t[:, sc[0]:sc[0] + 1],
                                                consts_t[:, sc[1]:sc[1] + 1],
                                                Alu.mult, Alu.add)
                        t_t = dpool.tile([DIM, wd], mybir.dt.float16,
                                         tag="t", padded_shape=[DIM, MAX_WD])
                        nc.vector.tensor_tensor(t_t[:], z_t[:], z_t[:],
                                                Alu.mult)
                        u_t = upool.tile([DIM, wd], mybir.dt.uint16,
                                         tag="u", padded_shape=[DIM, MAX_WD])
                        nc.vector.tensor_scalar(u_t[:], t_t[:], -1.0, B_EXP,
                                                Alu.mult, Alu.add)
                        if dist == "n":
                            un_t = u_t
                        else:
                            ua_t = u_t

                # --- matmuls in global chunk order ---
                for c in range(ca):
                    sl = slice(c * CHUNK, (c + 1) * CHUNK)
                    mm((sn_psA, sn_psB), wf_t, 128, en_t[:, sl], g)
                    mm((sa_psA, sa_psB), wf_t, 384, ea_t[:, sl], g)
                    g += 1
                for c in range(cd):
                    sl = slice(c * CHUNK, (c + 1) * CHUNK)
                    mm((sn_psA, sn_psB), wb_t, 128,
                       un_t[:, sl].bitcast(mybir.dt.bfloat16), g)
                    mm((sa_psA, sa_psB), wb_t, 384,
                       ua_t[:, sl].bitcast(mybir.dt.bfloat16), g)
                    g += 1

                if ti == FLUSH_AFTER_TILE:
                    # bank A complete: drain it under the remaining compute
                    sn_sbA = cpool.tile([64, CHUNK], mybir.dt.float32)
                    nc.vector.tensor_copy(sn_sbA[:], sn_psA[:])
                    sa_sbA = cpool.tile([64, CHUNK], mybir.dt.float32)
                    nc.vector.tensor_copy(sa_sbA[:], sa_psA[:])
                    nc.sync.dma_start(sn_out[0:64, :], sn_sbA[:])
                    nc.sync.dma_start(sa_out[0:64, :], sa_sbA[:])
                off += w

            # drain bank B on two different engines so the copies overlap
            sn_sbB = cpool.tile([64, CHUNK], mybir.dt.float32)
            nc.vector.tensor_copy(sn_sbB[:], sn_psB[:])
            sa_sbB = cpool.tile([64, CHUNK], mybir.dt.float32)
            nc.scalar.copy(sa_sbB[:], sa_psB[:])
            nc.sync.dma_start(sn_out[64:128, :], sn_sbB[:])
            nc.sync.dma_start(sa_out[64:128, :], sa_sbB[:])

    nc.compile()
    return nc


def _get_compiled():
    global _COMPILED
    if _COMPILED is None:
        _COMPILED = _build()
    return _COMPILED


def kernel(encoded, normal_dist, anomaly_dist):
    global LAST_RESULTS
    from concourse.bass_utils import run_bass_kernel_spmd

    x = np.asarray(encoded, dtype=np.float32)
    nd = np.asarray(normal_dist, dtype=np.float64)
    ad = np.asarray(anomaly_dist, dtype=np.float64)

    # per-dim stats (torch defaults: unbiased std)
    mu_n = nd.mean(axis=1)
    sd_n = nd.std(axis=1, ddof=1)
    mu_a = ad.mean(axis=1)
    sd_a = ad.std(axis=1, ddof=1)
    isd_n, isd_a = 1.0 / sd_n, 1.0 / sd_a

    inv_sqrt2 = 1.0 / np.sqrt(2.0)
    consts = np.stack([
        isd_n * inv_sqrt2,            # scale_n (ACT)
        -mu_n * isd_n * inv_sqrt2,    # bias_n
        isd_a * inv_sqrt2,            # scale_a
        -mu_a * isd_a * inv_sqrt2,    # bias_a
        A2 * isd_n,                   # s'_n (DVE)
        -A2 * mu_n * isd_n,           # b'_n
        A2 * isd_a,                   # s'_a
        -A2 * mu_a * isd_a,           # b'_a
    ], axis=1).astype(np.float32)     # [128, 8]

    half_sqrt_pi = 0.5 * np.sqrt(np.pi)
    c_n = (INV_SQRT_2PI * isd_n * half_sqrt_pi).astype(np.float32)
    c_a = (INV_SQRT_2PI * isd_a * half_sqrt_pi).astype(np.float32)
    wmat = np.zeros((DIM, 512), dtype=np.float32)
    wmat[:, 128] = c_n
    wmat[:, 384] = c_a
    import ml_dtypes
    wf16 = wmat.astype(np.float16)
    wbf16 = wmat.astype(ml_dtypes.bfloat16)

    xh = x.astype(np.float16)
    in_maps = []
    for i in range(NCORES):
        lo = i * R
        hi = min(lo + R, N)
        shard_T = np.zeros((DIM, R), dtype=np.float16)
        shard_T[:, :hi - lo] = xh[lo:hi].T
        in_maps.append({"xT": shard_T, "consts": consts,
                        "wf16": wf16, "wbf16": wbf16})

    nc = _get_compiled()
    try:
        res = run_bass_kernel_spmd(nc, in_maps, core_ids=list(range(NCORES)))
    except Exception:
        # one retry: the NRT occasionally reports a transient
        # NRT_EXEC_UNIT_UNRECOVERABLE on an otherwise-healthy device
        res = run_bass_kernel_spmd(nc, in_maps, core_ids=list(range(NCORES)))
    LAST_RESULTS = res

    s_n = np.empty(N, dtype=np.float64)
    s_a = np.empty(N, dtype=np.float64)
    for i in range(NCORES):
        lo = i * R
        hi = min(lo + R, N)
        s_n[lo:hi] = res.results[i]["sn_out"].reshape(-1)[:hi - lo]
        s_a[lo:hi] = res.results[i]["sa_out"].reshape(-1)[:hi - lo]

    # exact recurrence p_k = (p_{k-1} + s_k)/dim as truncated causal
    # convolution: p_k = sum_j (1/dim)^(j+1) s_{k-j}; (1/128)^14 ~ 3e-30.
    a = 1.0 / DIM
    pn = np.zeros(N, dtype=np.float64)
    pa = np.zeros(N, dtype=np.float64)
    wgt = a
    for j in range(14):
        if j == 0:
            pn += wgt * s_n
            pa += wgt * s_a
        else:
            pn[j:] += wgt * s_n[:-j]
            pa[j:] += wgt * s_a[:-j]
        wgt *= a
    total = pn + pa
    out = np.empty((N, 2), dtype=np.float32)
    out[:, 0] = (pn / total).astype(np.float32)
    out[:, 1] = (pa / total).astype(np.float32)
    return out
